# revision 1
# baseline (speedup 1.0000x reference)
"""Trainium2 Bass kernel for nn_MemoryEfficientBSpline (linear B-spline / KAN layer).

Math: out[b,o,p] = sum_i sum_g Wt[b,i,p,g] * coef[b,o,i,g] where Wt is the
two-hot linear-interpolation weight of x[b,i,p] over a 6-knot grid on [-1,1].

Reformulation (hinge basis): with xc = clip(x,-1,1) and nx = 2.5*(xc+1) in [0,5],
the piecewise-linear spline f(nx) = sum_k beta_k * relu(nx - k) + c0 becomes

  out[b,o,p] = alpha[b,o] + sum_i W0[b,o,i]*xc[b,i,p]
             + sum_{k=1..4} sum_i Wk[b,o,i]*relu(xc[b,i,p] + 1 - 0.4k)

i.e. 5 dense [64x64] matmuls over cheap elementwise "hinge planes" of x.
The one-hot construction disappears entirely; coefficients are transformed
host-side (tiny: [8,64,64,6]).

Sharding: data-parallel over batch B=8, one batch per NeuronCore. Per core the
64x36864 pixel plane is folded to 128 partitions (two pixel-halves stacked), and
the 64x64 weights are block-diagonal-duplicated to 128x128 so one full-array
matmul handles both halves.

Dtype: planes/weights are float32r (fp32 bits, reduced-precision PE multiply,
1 cycle/row) -> ~1e-4 rel err, 4x faster than strict fp32 matmul.
"""
import numpy as np
from contextlib import ExitStack

import concourse.bass as bass
import concourse.tile as tile
from concourse import bacc, mybir
from concourse.bass_utils import run_bass_kernel_spmd

# Problem shapes (hardcoded per contract)
B, IN_DIM, H, W = 8, 64, 192, 192
OUT_DIM = 64
G = 6
P_TOT = H * W          # 36864 pixels
HALF = P_TOT // 2      # 18432 (folded columns)
NPART = 128
SLAB = 2048            # columns processed per pipeline iteration
N_SLABS = HALF // SLAB # 9
CHUNK = 512            # matmul moving-operand size (= one PSUM bank of fp32)
N_CHUNKS = SLAB // CHUNK
NK = 5                 # hinge planes: xc, relu(xc+0.6), relu(xc+0.2), relu(xc-0.2), relu(xc-0.6)

_f32 = mybir.dt.float32
_f32r = mybir.dt.float32r
_Alu = mybir.AluOpType
_Act = mybir.ActivationFunctionType

_cached = None  # compiled Bass module, built once per process


def _build_module(n_reps=1):
    """n_reps>1 wraps the whole body in a hardware loop — used only for
    slope-based HW timing (dispatch noise >> exec time in this env)."""
    nc = bacc.Bacc("TRN2", target_bir_lowering=False, debug=False,
                   enable_asserts=False, num_devices=8)

    x_t = nc.dram_tensor("x", (NPART, HALF), _f32, kind="ExternalInput")
    w_t = nc.dram_tensor("wts", (NK, NPART, NPART), _f32r, kind="ExternalInput")
    b_t = nc.dram_tensor("bias", (NPART, 1), _f32, kind="ExternalInput")
    y_t = nc.dram_tensor("y", (NPART, HALF), _f32, kind="ExternalOutput")

    with tile.TileContext(nc) as tc, ExitStack() as ctx:
        cpool = ctx.enter_context(tc.tile_pool(name="const", bufs=1))
        xpool = ctx.enter_context(tc.tile_pool(name="xin", bufs=4))
        ppool = ctx.enter_context(tc.tile_pool(name="planes", bufs=3))
        opool = ctx.enter_context(tc.tile_pool(name="oslab", bufs=4))
        psum = ctx.enter_context(tc.tile_pool(name="acc", bufs=2, space="PSUM"))

        # Constants: weights [128, 5*128] (block-diag per plane), biases
        wts = cpool.tile([NPART, NK * NPART], _f32r)
        for k in range(NK):
            nc.sync.dma_start(wts[:, k*NPART:(k+1)*NPART], w_t[k])
        bias = cpool.tile([NPART, 1], _f32)
        nc.sync.dma_start(bias[:], b_t[:])
        c_p4 = cpool.tile([NPART, 1], _f32)
        nc.vector.memset(c_p4[:], -0.6)  # ACT-computed plane: relu(x - 0.6) bias

        def body():
            # Software-pipelined: evict of slab s-1 is emitted AFTER the ACT
            # plane of slab s, so the strict-FIFO ACT queue never makes the
            # next slab's plane wait behind the previous slab's psum evict.
            pend = None  # (psum_tile, col0) awaiting evict
            for s in range(N_SLABS + 1):
                if s < N_SLABS:
                    col0 = s * SLAB
                    xt = xpool.tile([NPART, SLAB], _f32)
                    nc.sync.dma_start(xt[:], x_t[:, col0:col0 + SLAB])

                    # Hinge planes (float32r out). relu(xc+c) == clip(x,-c,1)+c,
                    # with the +c shift folded into the output bias host-side —
                    # so DVE planes read raw x directly (no xc dependency chain).
                    # Slab 0 is emitted in 512-col quarters so the first matmul
                    # chunk's inputs are ready ~6us sooner (shorter pipeline fill).
                    pieces = 4 if s == 0 else 1
                    pw = SLAB // pieces
                    planes = []
                    xc = ppool.tile([NPART, SLAB], _f32r, tag="xc")
                    planes.append(xc)
                    for k, cst in ((1, 0.6), (2, 0.2), (3, -0.2)):
                        planes.append(ppool.tile([NPART, SLAB], _f32r,
                                                 tag=f"p{k}", name=f"p{k}"))
                    p4 = ppool.tile([NPART, SLAB], _f32r, tag="p4")
                    for q in range(pieces):
                        sl = slice(q * pw, (q + 1) * pw)
                        nc.vector.tensor_scalar(xc[:, sl], xt[:, sl], 1.0, -1.0,
                                                _Alu.min, _Alu.max)
                        for j, cst in ((1, 0.6), (2, 0.2), (3, -0.2)):
                            nc.vector.tensor_scalar(planes[j][:, sl], xt[:, sl],
                                                    1.0, -cst, _Alu.min, _Alu.max)
                        nc.scalar.activation(p4[:, sl], xc[:, sl], _Act.Relu,
                                             bias=c_p4[:], scale=1.0)
                    planes.append(p4)

                if pend is not None:
                    # Evict + bias in one ACT pass over 4 psum banks, DMA out
                    acc_p, pcol0 = pend
                    ot = opool.tile([NPART, SLAB], _f32)
                    nc.scalar.activation(ot[:], acc_p[:], _Act.Identity, bias=bias[:], scale=1.0)
                    nc.sync.dma_start(y_t[:, pcol0:pcol0 + SLAB], ot[:])
                    pend = None

                if s < N_SLABS:
                    # Matmuls: 5 planes x 4 chunks accumulate into 4-bank psum
                    acc = psum.tile([NPART, SLAB], _f32)
                    for k in range(NK):
                        wk = wts[:, k*NPART:(k+1)*NPART]
                        for c in range(N_CHUNKS):
                            nc.tensor.matmul(acc[:, c*CHUNK:(c+1)*CHUNK], wk,
                                             planes[k][:, c*CHUNK:(c+1)*CHUNK],
                                             start=(k == 0), stop=(k == NK - 1))
                    pend = (acc, col0)

        for _ in range(n_reps):
            body()

    nc.compile()
    return nc


def _get_module():
    global _cached
    if _cached is None:
        _cached = _build_module()
    return _cached


def _prep_inputs(x, coef):
    """Host-side shard + coefficient transform. Returns in_maps for 8 cores."""
    x = np.ascontiguousarray(x, dtype=np.float32)
    c = np.asarray(coef, dtype=np.float64)            # [B, o, i, 6]
    d = np.diff(c, axis=-1)                           # [B, o, i, 5]
    beta = np.concatenate([d[..., :1], np.diff(d, axis=-1)], axis=-1)
    Wk = (2.5 * beta).astype(np.float32)              # [B, o, i, 5]
    Wk64 = Wk.astype(np.float64)
    # Device planes k=1..3 are clip(x,-c_k,1) = relu(xc+c_k) - c_k: fold the
    # +c_k shift into the bias (c_k = 1 - 0.4k).
    alpha = (c[..., 0].sum(axis=2) + Wk64[..., 0].sum(axis=2)
             + sum((1.0 - 0.4 * k) * Wk64[..., k].sum(axis=2) for k in (1, 2, 3))
             ).astype(np.float32)                     # [B, o]

    in_maps = []
    eye2 = np.eye(2, dtype=np.float32)
    for b in range(B):
        xb = x[b].reshape(IN_DIM, P_TOT)
        x_f = np.concatenate([xb[:, :HALF], xb[:, HALF:]], axis=0)  # [128, HALF]
        # lhsT[k][i, o] = Wk[b, o, i, k], block-diag duplicated to 128x128
        lhsT = np.einsum('oik->kio', Wk[b])           # [5, i, o]
        wts = np.kron(eye2, lhsT).astype(np.float32)  # [5, 128, 128]
        bias = np.tile(alpha[b], 2).reshape(NPART, 1).astype(np.float32)
        in_maps.append({
            "x": np.ascontiguousarray(x_f),
            "wts": np.ascontiguousarray(wts),
            "bias": bias,
        })
    return in_maps


def _assemble(results):
    out = np.empty((B, OUT_DIM, H, W), dtype=np.float32)
    for b in range(B):
        y_f = results[b]["y"]                          # [128, HALF]
        out[b] = np.concatenate([y_f[:OUT_DIM], y_f[OUT_DIM:]], axis=1).reshape(OUT_DIM, H, W)
    return out


def run(x, coef, **spmd_kwargs):
    """Run on 8 NeuronCores; returns (output, BassKernelResults)."""
    nc = _get_module()
    in_maps = _prep_inputs(x, coef)
    res = run_bass_kernel_spmd(nc, in_maps, core_ids=list(range(8)), **spmd_kwargs)
    return _assemble(res.results), res


def kernel(x, coef):
    out, _ = run(x, coef)
    return out



# revision 2
# speedup vs baseline: 2.1643x; 2.1643x over previous
"""Trainium2 Bass kernel for nn_MemoryEfficientBSpline (linear B-spline / KAN layer).

Math: out[b,o,p] = sum_i sum_g Wt[b,i,p,g] * coef[b,o,i,g] where Wt is the
two-hot linear-interpolation weight of x[b,i,p] over a 6-knot grid on [-1,1].

Reformulation (hinge basis): the piecewise-linear spline becomes 5 dense
[64x64] matmuls over clip planes of x:

  out[b,o,p] = alpha[b,o] + sum_{k=0..4} sum_i Wk[b,o,i]*clip(x[b,i,p], t_k, 1)

with knots t = [-1, -0.6, -0.2, 0.2, 0.6]; the hinge shift relu(xc-t) =
clip(x,t,1) - t is folded into alpha host-side. All 5 planes are a single
DVE tensor_scalar (min,max) op each — no ACT dependency chain.

Sharding: data-parallel over batch B=8, one batch per NeuronCore. Per core the
64x36864 pixel plane is folded to 128 partitions (two pixel-halves stacked), and
the 64x64 weights are block-diagonal-duplicated to 128x128 so one full-array
matmul handles both halves.

Dtype: fp16 end-to-end (x in, planes, weights, y out; f32 psum accumulate).
Halves HBM traffic vs f32 (the 8-core steady state is chip-HBM-bound) and
doubles DVE throughput (4x mode for 16-bit SBUF operands). Rel err ~2e-3.

Schedule: PE warm-up dummy matmuls (on a memset scratch tile) cover the
~2.5us first-DMA latency and the DVFS p-state ramp; slab sizes ramp
128/384/1024 then 8x2048 with a 512 tail for a short drain.
"""
import numpy as np
from contextlib import ExitStack

import concourse.bass as bass
import concourse.tile as tile
from concourse import bacc, mybir
from concourse.bass_utils import run_bass_kernel_spmd

# Problem shapes (hardcoded per contract)
B, IN_DIM, H, W = 8, 64, 192, 192
OUT_DIM = 64
G = 6
P_TOT = H * W          # 36864 pixels
HALF = P_TOT // 2      # 18432 (folded columns)
NPART = 128
CHUNK = 512            # matmul moving-operand size (= one PSUM bank of fp32)
MAXSLAB = 2048
SLAB_SIZES = [128, 384, 1024] + [2048] * 8 + [512]   # sum = 18432
NK = 5                 # clip planes: t = -1, -0.6, -0.2, 0.2, 0.6
KNOTS = (-1.0, -0.6, -0.2, 0.2, 0.6)
N_DUMMY = 5            # PE warm-up matmuls (ramp + fill masking)

_f16 = mybir.dt.float16
_f32 = mybir.dt.float32
_Alu = mybir.AluOpType
_Act = mybir.ActivationFunctionType

_cached = None  # compiled Bass module, built once per process


def _build_module(n_reps=1):
    """n_reps>1 wraps the slab loop in-line — used only for slope-based HW
    timing (dispatch noise >> exec time in this env)."""
    nc = bacc.Bacc("TRN2", target_bir_lowering=False, debug=False,
                   enable_asserts=False, num_devices=8)

    x_t = nc.dram_tensor("x", (NPART, HALF), _f16, kind="ExternalInput")
    w_t = nc.dram_tensor("wts", (NPART, NK * NPART), _f16, kind="ExternalInput")
    b_t = nc.dram_tensor("bias", (NPART, 1), _f32, kind="ExternalInput")
    y_t = nc.dram_tensor("y", (NPART, HALF), _f16, kind="ExternalOutput")

    with tile.TileContext(nc) as tc, ExitStack() as ctx:
        cpool = ctx.enter_context(tc.tile_pool(name="const", bufs=1))
        xpool = ctx.enter_context(tc.tile_pool(name="xin", bufs=4))
        ppool = ctx.enter_context(tc.tile_pool(name="planes", bufs=3))
        opool = ctx.enter_context(tc.tile_pool(name="oslab", bufs=4))
        psum = ctx.enter_context(tc.tile_pool(name="acc", bufs=1, space="PSUM"))

        # Warm-up scratch: DVE memset is ready ~0.4us in, long before any DMA
        # lands, so dummy matmuls on it keep the PE busy (and ramping to full
        # clock) while the first x slab + weights are still in flight.
        warm = cpool.tile([NPART, CHUNK], _f16)
        nc.vector.memset(warm[:], 0.0)

        # Constants on the ACT queue so SP's first instruction is the slab-0
        # x load. k=0 weight block first: it gates the first real matmul.
        wts = cpool.tile([NPART, NK * NPART], _f16)
        nc.scalar.dma_start(wts[:, :NPART], w_t[:, :NPART])
        nc.scalar.dma_start(wts[:, NPART:], w_t[:, NPART:])
        bias = cpool.tile([NPART, 1], _f32)
        nc.scalar.dma_start(bias[:], b_t[:])

        acc0 = psum.tile([NPART, MAXSLAB], _f32, tag="acc0", name="acc0")
        acc1 = psum.tile([NPART, MAXSLAB], _f32, tag="acc1", name="acc1")
        accs = [acc0, acc1]

        for d in range(N_DUMMY):
            nc.tensor.matmul(acc0[:, :CHUNK], warm[:, :NPART], warm[:],
                             start=True, stop=True)

        n_slabs = len(SLAB_SIZES)
        col_starts = np.cumsum([0] + SLAB_SIZES).tolist()

        def body():
            for s in range(n_slabs):
                col0, sz = col_starts[s], SLAB_SIZES[s]
                xt = xpool.tile([NPART, MAXSLAB], _f16, tag="x", name="xt")
                nc.sync.dma_start(xt[:, :sz], x_t[:, col0:col0 + sz])

                # 5 clip planes, all independent single DVE ops on raw x
                planes = []
                for k, t in enumerate(KNOTS):
                    pk = ppool.tile([NPART, MAXSLAB], _f16, tag=f"p{k}",
                                    name=f"p{k}")
                    nc.vector.tensor_scalar(pk[:, :sz], xt[:, :sz], 1.0, t,
                                            _Alu.min, _Alu.max)
                    planes.append(pk)

                # Matmuls: 5 planes x 512-chunks accumulating in psum
                acc = accs[s % 2]
                for k in range(NK):
                    wk = wts[:, k * NPART:(k + 1) * NPART]
                    for c0 in range(0, sz, CHUNK):
                        w = min(CHUNK, sz - c0)
                        nc.tensor.matmul(acc[:, c0:c0 + w], wk,
                                         planes[k][:, c0:c0 + w],
                                         start=(k == 0), stop=(k == NK - 1))

                # Evict + bias in one ACT pass, then DMA out
                ot = opool.tile([NPART, MAXSLAB], _f16, tag="o", name="ot")
                nc.scalar.activation(ot[:, :sz], acc[:, :sz], _Act.Identity,
                                     bias=bias[:], scale=1.0)
                nc.sync.dma_start(y_t[:, col0:col0 + sz], ot[:, :sz])

        for _ in range(n_reps):
            body()

    nc.compile()
    return nc


def _get_module():
    global _cached
    if _cached is None:
        _cached = _build_module()
    return _cached


def _prep_inputs(x, coef):
    """Host-side shard + coefficient transform. Returns in_maps for 8 cores."""
    x16 = np.asarray(x, dtype=np.float16)             # [B, i, H, W]
    c = np.asarray(coef, dtype=np.float64)            # [B, o, i, 6]
    d = np.diff(c, axis=-1)                           # [B, o, i, 5]
    beta = np.concatenate([d[..., :1], np.diff(d, axis=-1)], axis=-1)
    Wk = (2.5 * beta).astype(np.float16)              # [B, o, i, 5]
    Wk64 = Wk.astype(np.float64)
    # Device plane k is clip(x, t_k, 1) = relu(xc - t_k) + t_k: fold the
    # +t_k shift into the output bias, using the fp16-rounded weights.
    alpha = (c[..., 0].sum(axis=2)
             - sum(t * Wk64[..., k].sum(axis=2) for k, t in enumerate(KNOTS))
             ).astype(np.float32)                     # [B, o]

    in_maps = []
    eye2 = np.eye(2, dtype=np.float16)
    for b in range(B):
        xb = x16[b].reshape(IN_DIM, P_TOT)
        x_f = np.concatenate([xb[:, :HALF], xb[:, HALF:]], axis=0)  # [128, HALF]
        # lhsT[k][i, o] = Wk[b, o, i, k], block-diag duplicated to 128x128,
        # packed as one contiguous [128, 5*128] tensor (single DMA)
        lhsT = np.einsum('oik->kio', Wk[b])           # [5, i, o]
        blocks = np.kron(eye2, lhsT)                  # [5, 128, 128]
        wts = np.ascontiguousarray(
            np.transpose(blocks, (1, 0, 2)).reshape(NPART, NK * NPART))
        bias = np.tile(alpha[b], 2).reshape(NPART, 1).astype(np.float32)
        in_maps.append({
            "x": np.ascontiguousarray(x_f),
            "wts": wts,
            "bias": bias,
        })
    return in_maps


def _assemble(results):
    out = np.empty((B, OUT_DIM, H, W), dtype=np.float32)
    for b in range(B):
        y_f = results[b]["y"].astype(np.float32)       # [128, HALF]
        out[b] = np.concatenate([y_f[:OUT_DIM], y_f[OUT_DIM:]], axis=1).reshape(OUT_DIM, H, W)
    return out


def run(x, coef, **spmd_kwargs):
    """Run on 8 NeuronCores; returns (output, BassKernelResults)."""
    nc = _get_module()
    in_maps = _prep_inputs(x, coef)
    res = run_bass_kernel_spmd(nc, in_maps, core_ids=list(range(8)), **spmd_kwargs)
    return _assemble(res.results), res


def kernel(x, coef):
    out, _ = run(x, coef)
    return out


# revision 23
# speedup vs baseline: 2.7370x; 1.2646x over previous
"""Trainium2 Bass kernel for nn_MemoryEfficientBSpline (linear B-spline / KAN layer).

Math: out[b,o,p] = sum_i sum_g Wt[b,i,p,g] * coef[b,o,i,g] where Wt is the
two-hot linear-interpolation weight of x[b,i,p] over a 6-knot grid on [-1,1].

Reformulation (hinge basis): the piecewise-linear spline becomes 5 dense
[64x64] matmuls over clip planes of x:

  out[b,o,p] = alpha[b,o] + sum_{k=0..4} sum_i Wk[b,o,i]*clip(x[b,i,p], t_k, 1)

with knots t = [-1, -0.6, -0.2, 0.2, 0.6]; the hinge shift relu(xc-t) =
clip(x,t,1) - t is folded into alpha host-side. All 5 planes are a single
DVE tensor_scalar (min,max) op each — no ACT dependency chain.

Sharding: data-parallel over batch B=8, one batch per NeuronCore. Per core the
64x36864 pixel plane is folded to 128 partitions (two pixel-halves stacked), and
the 64x64 weights are block-diagonal-duplicated to 128x128 so one full-array
matmul handles both halves.

Dtype: fp16 end-to-end (x in, planes, weights, y out; f32 psum accumulate).
Halves HBM traffic vs f32 (the 8-core steady state is chip-HBM-bound) and
doubles DVE throughput (4x mode for 16-bit SBUF operands). Rel err ~2e-3.

Schedule: PE warm-up dummy matmuls (on a memset scratch tile) cover the
~2.5us first-DMA latency and the DVFS p-state ramp; slab sizes ramp
128/384/1024 then 8x2048 with a 512 tail for a short drain.
"""
import numpy as np
from contextlib import ExitStack

import concourse.bass as bass
import concourse.tile as tile
from concourse import bacc, mybir
from concourse.bass_utils import run_bass_kernel_spmd

# Problem shapes (hardcoded per contract)
B, IN_DIM, H, W = 8, 64, 192, 192
OUT_DIM = 64
G = 6
P_TOT = H * W          # 36864 pixels
HALF = P_TOT // 2      # 18432 (folded columns)
NPART = 128
CHUNK = 512            # matmul moving-operand size (= one PSUM bank of fp32)
MAXSLAB = 2048
# Head ramps up so the first planes/matmuls start ASAP after the ~2.5us
# first-DMA latency; tail ramps down so the last evict+store drain is short.
SLAB_SIZES = [256, 512, 1024] + [2048] * 7 + [1024, 1024, 256]  # sum 18432
NK = 5                 # clip planes: t = -1, -0.6, -0.2, 0.2, 0.6
KNOTS = (-1.0, -0.6, -0.2, 0.2, 0.6)
N_DUMMY = 6            # PE warm-up matmuls (ramp + fill masking; any PE idle
                       # gap before the first real matmul resets the p-state
                       # ramp, so slightly over-covering the fill is cheaper)

_f16 = mybir.dt.float16
_f32 = mybir.dt.float32
_Alu = mybir.AluOpType
_Act = mybir.ActivationFunctionType

_cached = None  # compiled Bass module, built once per process


def _build_module(n_reps=1):
    """n_reps>1 wraps the slab loop in-line — used only for slope-based HW
    timing (dispatch noise >> exec time in this env)."""
    nc = bacc.Bacc("TRN2", target_bir_lowering=False, debug=False,
                   enable_asserts=False, num_devices=8)

    x_t = nc.dram_tensor("x", (NPART, HALF), _f16, kind="ExternalInput")
    w_t = nc.dram_tensor("wts", (NPART, NK * NPART), _f16, kind="ExternalInput")
    b_t = nc.dram_tensor("bias", (NPART, 1), _f32, kind="ExternalInput")
    y_t = nc.dram_tensor("y", (NPART, HALF), _f16, kind="ExternalOutput")

    with tile.TileContext(nc) as tc, ExitStack() as ctx:
        cpool = ctx.enter_context(tc.tile_pool(name="const", bufs=1))
        xpool = ctx.enter_context(tc.tile_pool(name="xin", bufs=4))
        ppool = ctx.enter_context(tc.tile_pool(name="planes", bufs=3))
        opool = ctx.enter_context(tc.tile_pool(name="oslab", bufs=4))
        psum = ctx.enter_context(tc.tile_pool(name="acc", bufs=1, space="PSUM"))

        # Warm-up scratch: DVE memset is ready ~0.4us in, long before any DMA
        # lands, so dummy matmuls on it keep the PE busy (and ramping to full
        # clock) while the first x slab + weights are still in flight.
        warm = cpool.tile([NPART, CHUNK], _f16)
        nc.vector.memset(warm[:], 0.0)

        # HWDGE configs serialize globally, so DMA issue order is critical:
        # weights first as ONE transfer (its sem gates every matmul; a split
        # serializes configs and delays k>=1 matmuls ~1.5us), slab-0 x second,
        # bias via the Pool queue's software DGE, which skips HWDGE entirely
        # (it's only needed by the first evict, ~8us in).
        wts = cpool.tile([NPART, NK * NPART], _f16)
        nc.sync.dma_start(wts[:], w_t[:])
        bias = cpool.tile([NPART, 1], _f32)
        nc.gpsimd.dma_start(bias[:], b_t[:])

        acc0 = psum.tile([NPART, MAXSLAB], _f32, tag="acc0", name="acc0")
        acc1 = psum.tile([NPART, MAXSLAB], _f32, tag="acc1", name="acc1")
        accs = [acc0, acc1]

        for d in range(N_DUMMY):
            nc.tensor.matmul(acc0[:, :CHUNK], warm[:, :NPART], warm[:],
                             start=True, stop=True)

        n_slabs = len(SLAB_SIZES)
        col_starts = np.cumsum([0] + SLAB_SIZES).tolist()
        col_starts.append(col_starts[-1])  # sentinel for load_slab(s+1) slice

        PREFETCH = 2    # slabs of load lead over the store stream on SP

        def load_slab(s):
            xt = xpool.tile([NPART, MAXSLAB], _f16, tag="x", name="xt")
            nc.sync.dma_start(xt[:, :SLAB_SIZES[s]],
                              x_t[:, col_starts[s]:col_starts[s + 1]])
            return xt

        def body():
            pending = [load_slab(s) for s in range(PREFETCH)]
            for s in range(n_slabs):
                col0, sz = col_starts[s], SLAB_SIZES[s]
                if s + PREFETCH < n_slabs:
                    pending.append(load_slab(s + PREFETCH))
                xt = pending.pop(0)

                # 5 clip planes, all independent single DVE ops on raw x
                planes = []
                for k, t in enumerate(KNOTS):
                    pk = ppool.tile([NPART, MAXSLAB], _f16, tag=f"p{k}",
                                    name=f"p{k}")
                    nc.vector.tensor_scalar(pk[:, :sz], xt[:, :sz], 1.0, t,
                                            _Alu.min, _Alu.max)
                    planes.append(pk)

                # Matmuls: 5 planes x 512-chunks accumulating in psum
                acc = accs[s % 2]
                for k in range(NK):
                    wk = wts[:, k * NPART:(k + 1) * NPART]
                    for c0 in range(0, sz, CHUNK):
                        w = min(CHUNK, sz - c0)
                        nc.tensor.matmul(acc[:, c0:c0 + w], wk,
                                         planes[k][:, c0:c0 + w],
                                         start=(k == 0), stop=(k == NK - 1))

                # Evict + bias in one ACT pass, then DMA out on SP. (Issuing
                # the store from ACT looks tempting but its DMA config blocks
                # the next evict on the ACT sequencer — measured worse.) The
                # LAST slab evicts on DVE instead: DVE is idle by then (all
                # planes done), while ACT still has the previous slab's evict
                # in flight — the two tail evicts run in parallel.
                ot = opool.tile([NPART, MAXSLAB], _f16, tag="o", name="ot")
                if s == n_slabs - 1:
                    nc.vector.tensor_scalar(ot[:, :sz], acc[:, :sz], bias[:],
                                            None, _Alu.add)
                else:
                    nc.scalar.activation(ot[:, :sz], acc[:, :sz], _Act.Identity,
                                         bias=bias[:], scale=1.0)
                nc.sync.dma_start(y_t[:, col0:col0 + sz], ot[:, :sz])

        for r in range(n_reps):
            body()

    nc.compile()
    return nc


def _get_module():
    global _cached
    if _cached is None:
        _cached = _build_module()
    return _cached


def _prep_inputs(x, coef):
    """Host-side shard + coefficient transform. Returns in_maps for 8 cores."""
    x16 = np.asarray(x, dtype=np.float16)             # [B, i, H, W]
    c = np.asarray(coef, dtype=np.float64)            # [B, o, i, 6]
    d = np.diff(c, axis=-1)                           # [B, o, i, 5]
    beta = np.concatenate([d[..., :1], np.diff(d, axis=-1)], axis=-1)
    Wk = (2.5 * beta).astype(np.float16)              # [B, o, i, 5]
    Wk64 = Wk.astype(np.float64)
    # Device plane k is clip(x, t_k, 1) = relu(xc - t_k) + t_k: fold the
    # +t_k shift into the output bias, using the fp16-rounded weights.
    alpha = (c[..., 0].sum(axis=2)
             - sum(t * Wk64[..., k].sum(axis=2) for k, t in enumerate(KNOTS))
             ).astype(np.float32)                     # [B, o]

    in_maps = []
    eye2 = np.eye(2, dtype=np.float16)
    for b in range(B):
        xb = x16[b].reshape(IN_DIM, P_TOT)
        x_f = np.concatenate([xb[:, :HALF], xb[:, HALF:]], axis=0)  # [128, HALF]
        # lhsT[k][i, o] = Wk[b, o, i, k], block-diag duplicated to 128x128,
        # packed as one contiguous [128, 5*128] tensor (single DMA)
        lhsT = np.einsum('oik->kio', Wk[b])           # [5, i, o]
        blocks = np.kron(eye2, lhsT)                  # [5, 128, 128]
        wts = np.ascontiguousarray(
            np.transpose(blocks, (1, 0, 2)).reshape(NPART, NK * NPART))
        bias = np.tile(alpha[b], 2).reshape(NPART, 1).astype(np.float32)
        in_maps.append({
            "x": np.ascontiguousarray(x_f),
            "wts": wts,
            "bias": bias,
        })
    return in_maps


def _assemble(results):
    out = np.empty((B, OUT_DIM, H, W), dtype=np.float32)
    for b in range(B):
        y_f = results[b]["y"].astype(np.float32)       # [128, HALF]
        out[b] = np.concatenate([y_f[:OUT_DIM], y_f[OUT_DIM:]], axis=1).reshape(OUT_DIM, H, W)
    return out


def run(x, coef, **spmd_kwargs):
    """Run on 8 NeuronCores; returns (output, BassKernelResults)."""
    nc = _get_module()
    in_maps = _prep_inputs(x, coef)
    res = run_bass_kernel_spmd(nc, in_maps, core_ids=list(range(8)), **spmd_kwargs)
    return _assemble(res.results), res


def kernel(x, coef):
    out, _ = run(x, coef)
    return out
